# revision 40
# baseline (speedup 1.0000x reference)
"""Tree-routed conditional matmul (MoE-style routing) on 8 TRN2 NeuronCores.

Key algebraic insight: the reference walks a depth-4 binary tree where the
direction at node n is sign(sum(x @ W[n])) = sign(x . Wsum[n]) with
Wsum[n] = W[n].sum(axis=1).  So the entire routing reduces to a tiny
[B, D] @ [D, 15] score matmul + an integer tree walk, and the only heavy
compute that contributes to the output is ONE [B, D] @ [D, D] matmul per
sample against its leaf-parent node's matrix (nodes 7..14) -- 1/15th of the
reference's FLOPs.

Sharding: expert-parallel.  Samples are grouped on host by their routed leaf
node; core i computes the dense matmul for group i (node 7+i) padded to a
fixed capacity C.  Routing + gather/scatter are part of the host-side
shard/unshard glue; the dense GEMMs run on the NeuronCores.
"""

import os
import sys

sys.path.insert(0, "/opt/trn_rl_repo")

import numpy as np

B = 4096
D = 2048
N_CORES = 8
P = 128  # partition size
NT = 4  # output column chunks of 512
NCHUNK = 512
KT = D // P  # 16 contraction tiles
C = 640  # per-core row capacity (multiple of 128); ~512 expected per group

# matmul input dtype: "f32" (exact, 4 cyc/row), "f32r" (fp32 storage,
# reduced-precision multiply, 1 cyc/row), "bf16" (1 cyc/row)
MM_DTYPE = os.environ.get("KERNEL_MM_DTYPE", "fp16")
VARIANT = os.environ.get("KERNEL_VARIANT", "v9")

_CACHE = {}


def _build_program(mm_dtype: str, variant: str | None = None):
    import concourse.bass as bass
    import concourse.tile as tile
    from concourse import bacc, mybir

    dt_map = {
        "f32": mybir.dt.float32,
        "f32r": mybir.dt.float32r,
        "bf16": mybir.dt.bfloat16,
        "fp16": mybir.dt.float16,
    }
    dt = dt_map[mm_dtype]
    variant = variant or VARIANT
    MT = C // P

    nc = bacc.Bacc("TRN2", target_bir_lowering=False, debug=False)
    xt_d = nc.dram_tensor("xt", [D, C], dt, kind="ExternalInput")
    w_d = nc.dram_tensor("w", [D, D], dt, kind="ExternalInput")
    out_d = nc.dram_tensor("out", [C, D], mybir.dt.float32, kind="ExternalOutput")

    with tile.TileContext(nc) as tc:
        with (
            tc.tile_pool(name="xtp", bufs=1) as xt_pool,
            tc.tile_pool(
                name="wp", bufs=(3 if variant in ("v3", "v4", "v6") else 2)
            ) as w_pool,
            tc.tile_pool(
                name="ps", bufs=(4 if variant == "v1" else 1), space="PSUM"
            ) as psum_pool,
            tc.tile_pool(name="op", bufs=4) as out_pool,
        ):
            if variant == "v1":
                xt_sb = xt_pool.tile([P, KT, C], dt)
                for k in range(KT):
                    nc.sync.dma_start(
                        xt_sb[:, k, :], xt_d.ap()[k * P : (k + 1) * P, :]
                    )
                for n in range(NT):
                    w_sb = w_pool.tile([P, KT, NCHUNK], dt)
                    for k in range(KT):
                        nc.sync.dma_start(
                            w_sb[:, k, :],
                            w_d.ap()[
                                k * P : (k + 1) * P, n * NCHUNK : (n + 1) * NCHUNK
                            ],
                        )
                    for m in range(MT):
                        ps = psum_pool.tile([P, NCHUNK], mybir.dt.float32)
                        for k in range(KT):
                            nc.tensor.matmul(
                                ps[:],
                                xt_sb[:, k, m * P : (m + 1) * P],
                                w_sb[:, k, :],
                                start=(k == 0),
                                stop=(k == KT - 1),
                            )
                        ot = out_pool.tile([P, NCHUNK], mybir.dt.float32)
                        nc.vector.tensor_copy(ot[:], ps[:])
                        nc.sync.dma_start(
                            out_d.ap()[
                                m * P : (m + 1) * P, n * NCHUNK : (n + 1) * NCHUNK
                            ],
                            ot[:],
                        )
            elif variant == "v2":
                # k-outer accumulation: compute starts as soon as the first
                # k-slices land; per-k tiles so deps are fine-grained.
                xt_k = []
                for k in range(KT):
                    t = xt_pool.tile([P, C], dt, tag=f"xt{k}")
                    nc.sync.dma_start(t[:], xt_d.ap()[k * P : (k + 1) * P, :])
                    xt_k.append(t)
                for n in range(NT):
                    w_k = []
                    for k in range(KT):
                        t = w_pool.tile([P, NCHUNK], dt, tag=f"w{k}")
                        nc.sync.dma_start(
                            t[:],
                            w_d.ap()[
                                k * P : (k + 1) * P, n * NCHUNK : (n + 1) * NCHUNK
                            ],
                        )
                        w_k.append(t)
                    pss = [
                        psum_pool.tile(
                            [P, NCHUNK], mybir.dt.float32, tag=f"ps{m}", name=f"ps{m}"
                        )
                        for m in range(MT)
                    ]
                    for k in range(KT):
                        for m in range(MT):
                            nc.tensor.matmul(
                                pss[m][:],
                                xt_k[k][:, m * P : (m + 1) * P],
                                w_k[k][:],
                                start=(k == 0),
                                stop=(k == KT - 1),
                            )
                    for m in range(MT):
                        ot = out_pool.tile([P, NCHUNK], mybir.dt.float32)
                        nc.vector.tensor_copy(ot[:], pss[m][:])
                        nc.sync.dma_start(
                            out_d.ap()[
                                m * P : (m + 1) * P, n * NCHUNK : (n + 1) * NCHUNK
                            ],
                            ot[:],
                        )
            elif variant == "v10":
                # m-outer/k-inner everywhere (staggered psum completion, smooth
                # slot recycling); PE head bubble filled by warm-up matmuls;
                # xt on scalar HWDGE ring, w on sync ring (w0 per-k, w1..3 as
                # merged halves); stores on gpsimd SWDGE, last chunk on scalar.
                wu_l = xt_pool.tile([P, P], dt, tag="wu_l", name="wu_l")
                wu_r = xt_pool.tile([P, NCHUNK], dt, tag="wu_r", name="wu_r")
                nc.gpsimd.memset(wu_l[:], 0.0)
                nc.gpsimd.memset(wu_r[:], 0.0)
                wu_ps = psum_pool.tile(
                    [P, NCHUNK], mybir.dt.float32, tag="ps4", name="wu_ps"
                )
                for i in range(14):
                    nc.tensor.matmul(
                        wu_ps[:], wu_l[:], wu_r[:], start=(i == 0), stop=(i == 13)
                    )
                xt_k = []
                w_chunk = {}
                for k in range(KT):
                    t = xt_pool.tile([P, C], dt, tag=f"xt{k}")
                    nc.scalar.dma_start(t[:], xt_d.ap()[k * P : (k + 1) * P, :])
                    xt_k.append(t)
                    wt = w_pool.tile(
                        [P, NCHUNK], dt, tag=f"w{k}", name=f"w0_{k}", bufs=1
                    )
                    nc.sync.dma_start(wt[:], w_d.ap()[k * P : (k + 1) * P, 0:NCHUNK])
                    w_chunk[(0, k)] = wt
                w_r = w_d.ap().rearrange("(k p) n -> p k n", p=P)
                for n in range(1, NT):
                    for h in range(2):
                        wt = w_pool.tile(
                            [P, KT // 2, NCHUNK],
                            dt,
                            tag=f"wh{h}",
                            name=f"w{n}_h{h}",
                            bufs=3,
                        )
                        nc.sync.dma_start(
                            wt[:],
                            w_r[
                                :,
                                h * (KT // 2) : (h + 1) * (KT // 2),
                                n * NCHUNK : (n + 1) * NCHUNK,
                            ],
                        )
                        for k in range(KT // 2):
                            w_chunk[(n, h * (KT // 2) + k)] = wt[:, k, :]
                for n in range(NT):
                    for m in range(MT):
                        ps = psum_pool.tile(
                            [P, NCHUNK],
                            mybir.dt.float32,
                            tag=f"ps{m}",
                            name=f"ps{n}_{m}",
                            bufs=(2 if m < 3 else 1),
                        )
                        for k in range(KT):
                            nc.tensor.matmul(
                                ps[:],
                                xt_k[k][:, m * P : (m + 1) * P],
                                w_chunk[(n, k)][:],
                                start=(k == 0),
                                stop=(k == KT - 1),
                            )
                        ot = out_pool.tile(
                            [P, NCHUNK], mybir.dt.float32, tag="ot", name="ot"
                        )
                        nc.vector.tensor_copy(ot[:], ps[:])
                        st_eng = nc.scalar if n == NT - 1 else nc.gpsimd
                        st_eng.dma_start(
                            out_d.ap()[
                                m * P : (m + 1) * P,
                                n * NCHUNK : (n + 1) * NCHUNK,
                            ],
                            ot[:],
                        )
            elif variant in ("v3", "v4", "v6", "v7", "v8", "v9"):
                xt_eng = nc.scalar if variant in ("v6", "v7", "v8", "v9") else nc.sync
                out_eng = (
                    nc.gpsimd
                    if variant in ("v7", "v8", "v9")
                    else (nc.scalar if variant == "v6" else nc.sync)
                )
                if variant == "v9":
                    # PE warm-up: fill the head DMA bubble with dummy matmuls
                    # so the HAM clock-gate flips to 2.4 GHz before real work.
                    wu_l = xt_pool.tile([P, P], dt, tag="wu_l", name="wu_l")
                    wu_r = xt_pool.tile([P, NCHUNK], dt, tag="wu_r", name="wu_r")
                    nc.gpsimd.memset(wu_l[:], 0.0)
                    nc.gpsimd.memset(wu_r[:], 0.0)
                    wu_ps = psum_pool.tile(
                        [P, NCHUNK], mybir.dt.float32, tag="ps4", name="wu_ps"
                    )
                    for i in range(9):
                        nc.tensor.matmul(
                            wu_ps[:], wu_l[:], wu_r[:], start=(i == 0), stop=(i == 8)
                        )
                # Interleave xt and first w-chunk DMAs so the k=0 tiles land
                # first and PE starts ~2us in.  v3: k-outer everywhere.
                # v4: k-outer for chunk 0 (hides head DMA), m-outer/k-inner for
                # chunks 1..3 (staggered psum completion -> stores overlap).
                xt_k = []
                w_chunk = {}
                for k in range(KT):
                    t = xt_pool.tile([P, C], dt, tag=f"xt{k}")
                    if variant == "v8" and k == 0:
                        xt_eng.dma_start(t[:, :P], xt_d.ap()[:P, :P])
                        xt_eng.dma_start(t[:, P:], xt_d.ap()[:P, P:])
                    else:
                        xt_eng.dma_start(t[:], xt_d.ap()[k * P : (k + 1) * P, :])
                    xt_k.append(t)
                    wt = w_pool.tile([P, NCHUNK], dt, tag=f"w{k}", name=f"w0_{k}")
                    nc.sync.dma_start(wt[:], w_d.ap()[k * P : (k + 1) * P, 0:NCHUNK])
                    w_chunk[(0, k)] = wt
                w_r = w_d.ap().rearrange("(k p) n -> p k n", p=P)  # [P, KT, D]
                for n in range(NT):
                    if n > 0 and variant in ("v7", "v8", "v9"):
                        # merged quarter-DMAs: fine-grained arrival, few events
                        NQ = 4 if variant == "v9" else 2
                        KQ = KT // NQ
                        for h in range(NQ):
                            wt = w_pool.tile(
                                [P, KQ, NCHUNK],
                                dt,
                                tag=f"wh{h}",
                                name=f"w{n}_h{h}",
                                bufs=3,
                            )
                            nc.sync.dma_start(
                                wt[:],
                                w_r[
                                    :,
                                    h * KQ : (h + 1) * KQ,
                                    n * NCHUNK : (n + 1) * NCHUNK,
                                ],
                            )
                            for k in range(KQ):
                                w_chunk[(n, h * KQ + k)] = wt[:, k, :]
                    elif n > 0:
                        for k in range(KT):
                            wt = w_pool.tile(
                                [P, NCHUNK], dt, tag=f"w{k}", name=f"w{n}_{k}"
                            )
                            nc.sync.dma_start(
                                wt[:],
                                w_d.ap()[
                                    k * P : (k + 1) * P,
                                    n * NCHUNK : (n + 1) * NCHUNK,
                                ],
                            )
                            w_chunk[(n, k)] = wt
                    if variant == "v3" or n == 0:
                        pss = [
                            psum_pool.tile(
                                [P, NCHUNK],
                                mybir.dt.float32,
                                tag=f"ps{m}",
                                name=f"ps{n}_{m}",
                                bufs=(2 if m < 3 else 1),
                            )
                            for m in range(MT)
                        ]
                        for k in range(KT):
                            for m in range(MT):
                                nc.tensor.matmul(
                                    pss[m][:],
                                    xt_k[k][:, m * P : (m + 1) * P],
                                    w_chunk[(n, k)][:],
                                    start=(k == 0),
                                    stop=(k == KT - 1),
                                )
                        for m in range(MT):
                            ot = out_pool.tile(
                                [P, NCHUNK], mybir.dt.float32, tag="ot", name="ot"
                            )
                            nc.vector.tensor_copy(ot[:], pss[m][:])
                            out_eng.dma_start(
                                out_d.ap()[
                                    m * P : (m + 1) * P,
                                    n * NCHUNK : (n + 1) * NCHUNK,
                                ],
                                ot[:],
                            )
                    else:
                        for m in range(MT):
                            st_eng = (
                                nc.scalar
                                if (variant in ("v8", "v9") and n == NT - 1)
                                else out_eng
                            )
                            last_tile = False  # split tried twice; net-negative in sim
                            # split the very last psum into two N/2 groups so
                            # its copy+store overlaps its own matmuls
                            halves = 2 if last_tile else 1
                            NH = NCHUNK // halves
                            for h in range(halves):
                                ps = psum_pool.tile(
                                    [P, NH],
                                    mybir.dt.float32,
                                    tag=f"ps{m}",
                                    name=f"ps{n}_{m}_{h}",
                                    bufs=(2 if m < 3 else 1),
                                )
                                for k in range(KT):
                                    nc.tensor.matmul(
                                        ps[:],
                                        xt_k[k][:, m * P : (m + 1) * P],
                                        w_chunk[(n, k)][:, h * NH : (h + 1) * NH],
                                        start=(k == 0),
                                        stop=(k == KT - 1),
                                    )
                                ot = out_pool.tile(
                                    [P, NH], mybir.dt.float32, tag="ot", name="ot"
                                )
                                nc.vector.tensor_copy(ot[:], ps[:])
                                st_eng.dma_start(
                                    out_d.ap()[
                                        m * P : (m + 1) * P,
                                        n * NCHUNK + h * NH : n * NCHUNK
                                        + (h + 1) * NH,
                                    ],
                                    ot[:],
                                )
            elif variant == "v5":
                # Two HWDGE rings: xt + out stores on scalar ring, w on sync
                # ring.  Chunk 0 fine-grained per-k; chunks 1..3 one merged
                # 3D DMA each.  k-outer chunk 0, m-outer/k-inner after.
                xt_k = []
                w_chunk = {}
                for k in range(KT):
                    t = xt_pool.tile([P, C], dt, tag=f"xt{k}")
                    nc.scalar.dma_start(t[:], xt_d.ap()[k * P : (k + 1) * P, :])
                    xt_k.append(t)
                    wt = w_pool.tile(
                        [P, NCHUNK], dt, tag=f"w0_{k}", name=f"w0_{k}", bufs=1
                    )
                    nc.sync.dma_start(wt[:], w_d.ap()[k * P : (k + 1) * P, 0:NCHUNK])
                    w_chunk[(0, k)] = wt
                w_r = w_d.ap().rearrange("(k p) n -> p k n", p=P)  # [128, KT, D]
                wn_tiles = {}
                for n in range(1, NT):
                    wt = w_pool.tile(
                        [P, KT, NCHUNK], dt, tag="wbig", name=f"wn{n}", bufs=3
                    )
                    nc.sync.dma_start(
                        wt[:], w_r[:, :, n * NCHUNK : (n + 1) * NCHUNK]
                    )
                    wn_tiles[n] = wt
                for n in range(NT):
                    if n == 0:
                        pss = [
                            psum_pool.tile(
                                [P, NCHUNK],
                                mybir.dt.float32,
                                tag=f"ps{m}",
                                name=f"ps0_{m}",
                                bufs=(2 if m < 3 else 1),
                            )
                            for m in range(MT)
                        ]
                        for k in range(KT):
                            for m in range(MT):
                                nc.tensor.matmul(
                                    pss[m][:],
                                    xt_k[k][:, m * P : (m + 1) * P],
                                    w_chunk[(0, k)][:],
                                    start=(k == 0),
                                    stop=(k == KT - 1),
                                )
                        for m in range(MT):
                            ot = out_pool.tile(
                                [P, NCHUNK], mybir.dt.float32, tag="ot", name="ot"
                            )
                            nc.vector.tensor_copy(ot[:], pss[m][:])
                            nc.scalar.dma_start(
                                out_d.ap()[m * P : (m + 1) * P, 0:NCHUNK], ot[:]
                            )
                    else:
                        wt = wn_tiles[n]
                        for m in range(MT):
                            ps = psum_pool.tile(
                                [P, NCHUNK],
                                mybir.dt.float32,
                                tag=f"ps{m}",
                                name=f"ps{n}_{m}",
                                bufs=(2 if m < 3 else 1),
                            )
                            for k in range(KT):
                                nc.tensor.matmul(
                                    ps[:],
                                    xt_k[k][:, m * P : (m + 1) * P],
                                    wt[:, k, :],
                                    start=(k == 0),
                                    stop=(k == KT - 1),
                                )
                            ot = out_pool.tile(
                                [P, NCHUNK], mybir.dt.float32, tag="ot", name="ot"
                            )
                            nc.vector.tensor_copy(ot[:], ps[:])
                            nc.scalar.dma_start(
                                out_d.ap()[
                                    m * P : (m + 1) * P,
                                    n * NCHUNK : (n + 1) * NCHUNK,
                                ],
                                ot[:],
                            )
            else:
                raise ValueError(variant)
    nc.compile()
    return nc


def _get_program(mm_dtype: str, variant: str | None = None):
    key = (mm_dtype, variant or VARIANT)
    if key not in _CACHE:
        _CACHE[key] = _build_program(mm_dtype, variant)
    return _CACHE[key]


def _route(x: np.ndarray, W: np.ndarray) -> np.ndarray:
    """Leaf-parent group id (0..7) per sample, computed in float64."""
    Wsum = W.sum(axis=2)  # [15, D]
    s = x.astype(np.float64) @ Wsum.T.astype(np.float64)  # [B, 15]
    node = np.zeros(x.shape[0], dtype=np.int64)
    rows = np.arange(x.shape[0])
    for _ in range(3):
        node = 2 * node + 1 + (s[rows, node] > 0).astype(np.int64)
    return node - 7


def kernel(x: np.ndarray, W: np.ndarray, _trace=False, _trace_kwargs=None):
    from concourse import bass_utils

    x = np.ascontiguousarray(np.asarray(x, dtype=np.float32))
    W = np.asarray(W, dtype=np.float32)

    g = _route(x, W)
    idxs = [np.where(g == i)[0] for i in range(N_CORES)]

    if MM_DTYPE == "bf16":
        import ml_dtypes

        host_dt = ml_dtypes.bfloat16
    elif MM_DTYPE == "fp16":
        host_dt = np.float16
    else:
        host_dt = np.float32

    in_maps = []
    overflow = []  # (core, rows beyond capacity) -> host fallback
    for i in range(N_CORES):
        idx = idxs[i]
        if len(idx) > C:
            overflow.append((i, idx[C:]))
            idx = idx[:C]
            idxs[i] = idx
        xt = np.zeros((D, C), dtype=host_dt)
        xt[:, : len(idx)] = x[idx].T
        in_maps.append(
            {"xt": xt, "w": np.ascontiguousarray(W[7 + i].astype(host_dt))}
        )

    nc = _get_program(MM_DTYPE)
    res = bass_utils.run_bass_kernel_spmd(
        nc,
        in_maps,
        core_ids=list(range(N_CORES)),
        trace=_trace,
        **(_trace_kwargs or {}),
    )

    out = np.empty((B, D), dtype=np.float32)
    for i in range(N_CORES):
        idx = idxs[i]
        out[idx] = res.results[i]["out"][: len(idx)]
    for i, extra in overflow:
        out[extra] = x[extra] @ W[7 + i]
    if _trace:
        return out, res
    return out
